# revision 14
# baseline (speedup 1.0000x reference)
"""Causal self-attention (B=4, L=2048, D=1024, H=16, HD=64) on 8 TRN2 cores.

Sharding: 8 shards = 4 batches x 2 head-groups (8 heads each). Each core:
  - QKV projection for its 8 heads (Q^T/K^T in [HD, L] layout, V in [L, HD])
  - causal attention per head, softmax without max-subtraction (logits are
    small by construction), row sums via a ones-column appended to V
  - partial output projection with its 512 rows of out_w
Host sums the two partials per batch and adds out_b.

v2 structure (vs v1): phases interleaved per 512-column tile so ScalarE exp
work overlaps projection/output-projection matmuls; the even/odd head S and
exp are fused ([128, 2, 512] PSUM tiles -> one exp instruction per key
block); normalize reads U directly from PSUM; output stored as bf16.

All matmuls run in bf16 (fp32 PSUM accumulation); exp on ScalarE in fp32.
"""

import os

import numpy as np
import ml_dtypes

B, L, D, H, HD = 4, 2048, 1024, 16, 64
HPC = 8           # heads per core
NCORES = 8
QT_TILE = 512     # q columns per attention tile
NKB = L // 128    # key blocks of 128

_STATE = {}


def _build_nc(repeat=1):
    import concourse.bass as bass
    import concourse.mybir as mybir
    import concourse.tile as tile
    from concourse import bacc
    from concourse.masks import make_upper_triangular

    f32 = mybir.dt.float32
    bf16 = mybir.dt.bfloat16
    AF = mybir.ActivationFunctionType
    OP = mybir.AluOpType

    nc = bacc.Bacc(None, target_bir_lowering=False)

    xT = nc.dram_tensor("xT", [D, L], bf16, kind="ExternalInput")
    wqk = nc.dram_tensor("wqk", [D, 2 * HPC * HD], bf16, kind="ExternalInput")
    wv = nc.dram_tensor("wv", [D, HPC * HD], bf16, kind="ExternalInput")
    bqk = nc.dram_tensor("bqk", [128, 8], f32, kind="ExternalInput")
    bv = nc.dram_tensor("bv", [1, HPC * HD], bf16, kind="ExternalInput")
    w2 = nc.dram_tensor("w2", [HPC * HD, D], bf16, kind="ExternalInput")
    out = nc.dram_tensor("out", [L, D], bf16, kind="ExternalOutput")

    KO = D // 128  # contraction blocks for the projections

    with tile.TileContext(nc) as tc:
        with (
            tc.tile_pool(name="const", bufs=1) as cpool,
            tc.tile_pool(name="weights", bufs=1) as wpool,
            tc.tile_pool(name="resident", bufs=1) as rpool,
            tc.tile_pool(name="xc", bufs=4) as xcpool,
            tc.tile_pool(name="ework", bufs=6) as epool,
            tc.tile_pool(name="ywork", bufs=3) as ypool,
            tc.tile_pool(name="rwork", bufs=3) as rwpool,
            tc.tile_pool(name="ps_mm", bufs=2, space="PSUM") as ps_mm,
            tc.tile_pool(name="ps_s", bufs=2, space="PSUM") as ps_s,
            tc.tile_pool(name="ps_u", bufs=1, space="PSUM") as ps_u,
        ):
            ones = cpool.tile([1, 128], bf16)
            nc.vector.memset(ones[:], 1.0)
            mask = cpool.tile([128, 128], bf16)
            make_upper_triangular(nc, mask[:], val=1.0, diag=True)
            # same mask replicated for the fused even/odd [128, 2, 128] op
            mask3 = cpool.tile([128, 2, 128], bf16)
            nc.vector.tensor_copy(mask3[:, 0, :], mask[:])
            nc.vector.tensor_copy(mask3[:, 1, :], mask[:])


            wqk_sb = wpool.tile([128, KO, 2 * HPC * HD], bf16)
            nc.sync.dma_start(wqk_sb[:], wqk.rearrange("(ko p) m -> p ko m", p=128))
            wv_sb = wpool.tile([128, KO, HPC * HD], bf16)
            nc.sync.dma_start(wv_sb[:], wv.rearrange("(ko p) m -> p ko m", p=128))
            bqk_sb = wpool.tile([128, 8], f32)
            nc.sync.dma_start(bqk_sb[:], bqk[:])
            bv_sb = wpool.tile([1, HPC * HD], bf16)
            nc.sync.dma_start(bv_sb[:], bv[:])
            w2_sb = wpool.tile([128, 4, D], bf16)
            nc.sync.dma_start(w2_sb[:], w2.rearrange("(o p) n -> p o n", p=128))

            # Q^T / K^T packed as head pairs: head h lives at partitions
            # (h%2)*64..+64 of block h//2. KT/V are double-buffered across
            # passes so pass i+1's projections overlap pass i's attention
            # tail instead of serializing on the WAR hazard.
            QT = rpool.tile([128, 4, L], bf16)
            OT = rpool.tile([128, 4, L], bf16)

            xTr = xT.rearrange("(ko p) n -> p ko n", p=128)

            def emit_proj(jt, KT, V):
                sl = slice(jt * 512, (jt + 1) * 512)
                xc = xcpool.tile([128, KO, 512], bf16, tag="xc")
                nc.sync.dma_start(xc[:], xTr[:, :, sl])
                # Q^T / K^T: out = wqk.T @ x^T
                for mb in range(8):
                    t = ps_mm.tile([128, 512], f32, tag="mm512")
                    for ko in range(KO):
                        nc.tensor.matmul(
                            t[:],
                            wqk_sb[:, ko, mb * 128:(mb + 1) * 128],
                            xc[:, ko, :],
                            start=(ko == 0),
                            stop=(ko == KO - 1),
                        )
                    dst = QT[:, mb, sl] if mb < 4 else KT[:, mb - 4, sl]
                    nc.vector.tensor_scalar_add(dst, t[:], bqk_sb[:, mb:mb + 1])
                # V: natural layout, bias folded in via a ones contraction row
                for qb in range(4):
                    g = jt * 4 + qb
                    tv = ps_mm.tile([128, 512], f32, tag="mm512")
                    for ko in range(KO):
                        nc.tensor.matmul(
                            tv[:],
                            xc[:, ko, qb * 128:(qb + 1) * 128],
                            wv_sb[:, ko, :],
                            start=(ko == 0),
                            stop=False,
                        )
                    nc.tensor.matmul(
                        tv[:], ones[0:1, :], bv_sb[0:1, :], start=False, stop=True
                    )
                    nc.vector.tensor_copy(
                        V[:, g, :, 0:64], tv.rearrange("p (h e) -> p h e", e=HD)
                    )

            def emit_attn(jt, KT, V):
                # attention for query tile jt over key blocks 0..(jt+1)*4-1;
                # even head on partitions 0-63, odd on 64-127 (their K=64
                # S^T matmuls target different PE row groups and overlap).
                for hp in range(4):
                    u = ps_u.tile([65, 2, 512], f32, tag="u")
                    nkb = (jt + 1) * (QT_TILE // 128)
                    for kb in range(nkb):
                        q_off = max(0, kb * 128 - jt * QT_TILE)
                        qsl = slice(jt * QT_TILE + q_off, (jt + 1) * QT_TILE)
                        ksl = slice(kb * 128, (kb + 1) * 128)
                        s = ps_s.tile([128, 2, 512], f32, tag="s")
                        nc.tensor.matmul(
                            s[:, 0, q_off:], KT[0:64, hp, ksl], QT[0:64, hp, qsl],
                            start=True, stop=True,
                        )
                        nc.tensor.matmul(
                            s[:, 1, q_off:], KT[64:128, hp, ksl], QT[64:128, hp, qsl],
                            start=True, stop=True,
                        )
                        et = epool.tile([128, 2, 512], bf16, tag="et")
                        nc.scalar.activation(
                            et[:, :, q_off:], s[:, :, q_off:], AF.Exp, scale=0.125)
                        if kb * 128 >= jt * QT_TILE:  # diagonal block
                            nc.vector.tensor_tensor(
                                out=et[:, :, q_off:q_off + 128],
                                in0=et[:, :, q_off:q_off + 128],
                                in1=mask3[:],
                                op=OP.mult,
                            )
                        nc.tensor.matmul(
                            u[:, 0, q_off:], V[:, kb, 2 * hp, 0:65],
                            et[:, 0, q_off:],
                            start=(kb == 0), stop=(kb == nkb - 1),
                        )
                        nc.tensor.matmul(
                            u[:, 1, q_off:], V[:, kb, 2 * hp + 1, 0:65],
                            et[:, 1, q_off:],
                            start=(kb == 0), stop=(kb == nkb - 1),
                        )
                    # normalize: O^T = U^T * (1/rowsum), rowsum broadcast
                    # along partitions via one PE rank-2 matmul for both heads
                    sl = slice(jt * QT_TILE, (jt + 1) * QT_TILE)
                    rcp_e = rwpool.tile([1, 512], bf16, tag="rcp_e")
                    rcp_o = rwpool.tile([1, 512], bf16, tag="rcp_o")
                    with nc.allow_low_precision(reason="bf16 recip feeds bf16 bcast"):
                        nc.vector.reciprocal(rcp_e[:], u[64:65, 0, :])
                        nc.vector.reciprocal(rcp_o[:], u[64:65, 1, :])
                    # two rank-1 broadcasts into disjoint PE column groups
                    b_ps = ps_mm.tile([128, 512], f32, tag="mm512")
                    nc.tensor.matmul(
                        b_ps[0:64, :], ones[0:1, 0:64], rcp_e[:],
                        start=True, stop=True)
                    nc.tensor.matmul(
                        b_ps[64:128, :], ones[0:1, 0:64], rcp_o[:],
                        start=True, stop=True)
                    b_sb = rwpool.tile([128, 512], f32, tag="b_sb")
                    nc.vector.tensor_copy(b_sb[:], b_ps[:])
                    nc.vector.tensor_tensor(
                        out=OT[0:64, hp, sl], in0=u[0:64, 0, :],
                        in1=b_sb[0:64, :], op=OP.mult)
                    nc.vector.tensor_tensor(
                        out=OT[64:128, hp, sl], in0=u[0:64, 1, :],
                        in1=b_sb[64:128, :], op=OP.mult)

            def emit_outproj(jt):
                for qb in range(jt * 4, (jt + 1) * 4):
                    for nb in range(D // 512):
                        y_ps = ps_mm.tile([128, 512], f32, tag="mm512")
                        for hp in range(4):
                            nc.tensor.matmul(
                                y_ps[:],
                                OT[:, hp, qb * 128:(qb + 1) * 128],
                                w2_sb[:, hp, nb * 512:(nb + 1) * 512],
                                start=(hp == 0),
                                stop=(hp == 3),
                            )
                        y_sb = ypool.tile([128, 512], bf16, tag="y_sb")
                        nc.vector.tensor_copy(y_sb[:], y_ps[:])
                        nc.sync.dma_start(
                            out[qb * 128:(qb + 1) * 128,
                                nb * 512:(nb + 1) * 512], y_sb[:]
                        )

            def emit_pass():
                phases = os.environ.get("KERNEL_PHASES", "ABC")
                KT = rpool.tile([128, 4, L], bf16, tag="KT", bufs=2)
                # V with a ones column at index 64 (col 65 is padding).
                V = rpool.tile([128, NKB, HPC, 66], bf16, tag="V", bufs=2)
                nc.vector.memset(V[:, :, :, 64:66], 0.0)
                nc.vector.memset(V[:, :, :, 64:65], 1.0)
                for jt in range(L // 512):
                    if "A" in phases:
                        emit_proj(jt, KT, V)
                    if "B" in phases:
                        emit_attn(jt, KT, V)
                    if "C" in phases:
                        emit_outproj(jt)

            for _rep in range(repeat):
                emit_pass()
    nc.compile()
    return nc


def _get_nc():
    if "nc" not in _STATE:
        _STATE["nc"] = _build_nc()
    return _STATE["nc"]


def kernel(x, in_w, in_b, out_w, out_b):
    from concourse.bass_utils import run_bass_kernel_spmd

    bf = ml_dtypes.bfloat16
    x = np.asarray(x, dtype=np.float32)
    in_w = np.asarray(in_w, dtype=np.float32)
    in_b = np.asarray(in_b, dtype=np.float32)
    out_w = np.asarray(out_w, dtype=np.float32)
    out_b = np.asarray(out_b, dtype=np.float32)

    nc = _get_nc()

    in_maps = []
    for c in range(NCORES):
        b, hg = c // 2, c % 2
        hsl = slice(hg * HPC * HD, (hg + 1) * HPC * HD)  # 512 cols of each section
        wq = in_w[:, 0:D][:, hsl]
        wk = in_w[:, D:2 * D][:, hsl]
        wv_ = in_w[:, 2 * D:3 * D][:, hsl]
        bq = in_b[0:D][hsl]
        bk = in_b[D:2 * D][hsl]
        bv_ = in_b[2 * D:3 * D][hsl]
        in_maps.append({
            "xT": np.ascontiguousarray(x[b].T).astype(bf),
            "wqk": np.ascontiguousarray(
                np.concatenate([wq, wk], axis=1)).astype(bf),
            "wv": np.ascontiguousarray(wv_).astype(bf),
            "bqk": np.ascontiguousarray(
                np.concatenate([bq, bk]).reshape(8, 128).T).astype(np.float32),
            "bv": np.ascontiguousarray(bv_.reshape(1, -1)).astype(bf),
            "w2": np.ascontiguousarray(out_w[hsl, :]).astype(bf),
        })

    trace = bool(int(os.environ.get("KERNEL_TRACE", "0")))
    if not trace:
        # the axon NTFF profile hook is absent in this container; make sure a
        # stray BASS_TRACE=1 in the environment can't route us into it
        os.environ["BASS_NEVER_TRACE"] = "1"
    res = run_bass_kernel_spmd(
        nc, in_maps, core_ids=list(range(NCORES)), trace=trace,
    )
    _STATE["last_result"] = res
    _STATE["last_in_maps"] = in_maps

    y = np.zeros((B, L, D), dtype=np.float32)
    for c in range(NCORES):
        y[c // 2] += res.results[c]["out"].astype(np.float32)
    y += out_b[None, None, :]
    return y


# revision 16
# speedup vs baseline: 1.1252x; 1.1252x over previous
"""Causal self-attention (B=4, L=2048, D=1024, H=16, HD=64) on 8 TRN2 cores.

Sharding: 8 shards = 4 batches x 2 head-groups (8 heads each). Each core:
  - QKV projection for its 8 heads (Q^T/K^T in [HD, L] layout, V in [L, HD])
  - causal attention per head, softmax without max-subtraction (logits are
    small by construction), row sums via a ones-column appended to V
  - partial output projection with its 512 rows of out_w
Host sums the two partials per batch and adds out_b.

v2 structure (vs v1): phases interleaved per 512-column tile so ScalarE exp
work overlaps projection/output-projection matmuls; the even/odd head S and
exp are fused ([128, 2, 512] PSUM tiles -> one exp instruction per key
block); normalize reads U directly from PSUM; output stored as bf16.

All matmuls run in bf16 (fp32 PSUM accumulation); exp on ScalarE in fp32.
"""

import os

import numpy as np
import ml_dtypes

B, L, D, H, HD = 4, 2048, 1024, 16, 64
HPC = 8           # heads per core
NCORES = 8
QT_TILE = 512     # q columns per attention tile
NKB = L // 128    # key blocks of 128

_STATE = {}


def _build_nc(repeat=1):
    import concourse.bass as bass
    import concourse.mybir as mybir
    import concourse.tile as tile
    from concourse import bacc
    from concourse.masks import make_upper_triangular

    f32 = mybir.dt.float32
    bf16 = mybir.dt.bfloat16
    AF = mybir.ActivationFunctionType
    OP = mybir.AluOpType

    nc = bacc.Bacc(None, target_bir_lowering=False)

    xT = nc.dram_tensor("xT", [D, L], bf16, kind="ExternalInput")
    wqk = nc.dram_tensor("wqk", [D, 2 * HPC * HD], bf16, kind="ExternalInput")
    wv = nc.dram_tensor("wv", [D, HPC * HD], bf16, kind="ExternalInput")
    bqk = nc.dram_tensor("bqk", [128, 8], f32, kind="ExternalInput")
    bv = nc.dram_tensor("bv", [1, HPC * HD], bf16, kind="ExternalInput")
    w2 = nc.dram_tensor("w2", [HPC * HD, D], bf16, kind="ExternalInput")
    out = nc.dram_tensor("out", [L, D], bf16, kind="ExternalOutput")

    KO = D // 128  # contraction blocks for the projections

    with tile.TileContext(nc) as tc:
        with (
            tc.tile_pool(name="const", bufs=1) as cpool,
            tc.tile_pool(name="weights", bufs=1) as wpool,
            tc.tile_pool(name="resident", bufs=1) as rpool,
            tc.tile_pool(name="xc", bufs=4) as xcpool,
            tc.tile_pool(name="ework", bufs=4) as epool,
            tc.tile_pool(name="ywork", bufs=3) as ypool,
            tc.tile_pool(name="rwork", bufs=2) as rwpool,
            tc.tile_pool(name="ps_mm", bufs=2, space="PSUM") as ps_mm,
            tc.tile_pool(name="ps_s", bufs=2, space="PSUM") as ps_s,
            tc.tile_pool(name="ps_u", bufs=1, space="PSUM") as ps_u,
        ):
            ones = cpool.tile([1, 128], bf16)
            nc.vector.memset(ones[:], 1.0)
            mask = cpool.tile([128, 128], bf16)
            make_upper_triangular(nc, mask[:], val=1.0, diag=True)
            # same mask replicated for the fused even/odd [128, 2, 128] op
            mask3 = cpool.tile([128, 2, 128], bf16)
            nc.vector.tensor_copy(mask3[:, 0, :], mask[:])
            nc.vector.tensor_copy(mask3[:, 1, :], mask[:])


            wqk_sb = wpool.tile([128, KO, 2 * HPC * HD], bf16)
            nc.sync.dma_start(wqk_sb[:], wqk.rearrange("(ko p) m -> p ko m", p=128))
            wv_sb = wpool.tile([128, KO, HPC * HD], bf16)
            nc.sync.dma_start(wv_sb[:], wv.rearrange("(ko p) m -> p ko m", p=128))
            bqk_sb = wpool.tile([128, 8], f32)
            nc.sync.dma_start(bqk_sb[:], bqk[:])
            bv_sb = wpool.tile([1, HPC * HD], bf16)
            nc.sync.dma_start(bv_sb[:], bv[:])
            w2_sb = wpool.tile([128, 4, D], bf16)
            nc.sync.dma_start(w2_sb[:], w2.rearrange("(o p) n -> p o n", p=128))

            # Q^T / K^T packed as head pairs: head h lives at partitions
            # (h%2)*64..+64 of block h//2.
            QT = rpool.tile([128, 4, L], bf16)
            KT = rpool.tile([128, 4, L], bf16)
            # V with a ones column at index 64 (col 65 is alignment padding).
            V = rpool.tile([128, NKB, HPC, 66], bf16)
            nc.vector.memset(V[:, :, :, 64:66], 0.0)
            nc.vector.memset(V[:, :, :, 64:65], 1.0)
            OT = rpool.tile([128, 4, L], bf16)

            xTr = xT.rearrange("(ko p) n -> p ko n", p=128)

            def emit_proj(jt):
                sl = slice(jt * 512, (jt + 1) * 512)
                xc = xcpool.tile([128, KO, 512], bf16, tag="xc")
                nc.sync.dma_start(xc[:], xTr[:, :, sl])
                # Q^T / K^T: out = wqk.T @ x^T
                for mb in range(8):
                    t = ps_mm.tile([128, 512], f32, tag="mm512")
                    for ko in range(KO):
                        nc.tensor.matmul(
                            t[:],
                            wqk_sb[:, ko, mb * 128:(mb + 1) * 128],
                            xc[:, ko, :],
                            start=(ko == 0),
                            stop=(ko == KO - 1),
                        )
                    dst = QT[:, mb, sl] if mb < 4 else KT[:, mb - 4, sl]
                    nc.vector.tensor_scalar_add(dst, t[:], bqk_sb[:, mb:mb + 1])
                # V: natural layout, bias folded in via a ones contraction row
                for qb in range(4):
                    g = jt * 4 + qb
                    tv = ps_mm.tile([128, 512], f32, tag="mm512")
                    for ko in range(KO):
                        nc.tensor.matmul(
                            tv[:],
                            xc[:, ko, qb * 128:(qb + 1) * 128],
                            wv_sb[:, ko, :],
                            start=(ko == 0),
                            stop=False,
                        )
                    nc.tensor.matmul(
                        tv[:], ones[0:1, :], bv_sb[0:1, :], start=False, stop=True
                    )
                    nc.vector.tensor_copy(
                        V[:, g, :, 0:64], tv.rearrange("p (h e) -> p h e", e=HD)
                    )

            def emit_attn(jt):
                # attention for query tile jt over key blocks 0..(jt+1)*4-1;
                # even head on partitions 0-63, odd on 64-127 (their K=64
                # S^T matmuls target different PE row groups and overlap).
                for hp in range(4):
                    u = ps_u.tile([65, 2, 512], f32, tag="u")
                    nkb = (jt + 1) * (QT_TILE // 128)
                    for kb in range(nkb):
                        q_off = max(0, kb * 128 - jt * QT_TILE)
                        qsl = slice(jt * QT_TILE + q_off, (jt + 1) * QT_TILE)
                        ksl = slice(kb * 128, (kb + 1) * 128)
                        s = ps_s.tile([128, 2, 512], f32, tag="s")
                        nc.tensor.matmul(
                            s[:, 0, q_off:], KT[0:64, hp, ksl], QT[0:64, hp, qsl],
                            start=True, stop=True,
                        )
                        nc.tensor.matmul(
                            s[:, 1, q_off:], KT[64:128, hp, ksl], QT[64:128, hp, qsl],
                            start=True, stop=True,
                        )
                        et = epool.tile([128, 2, 512], bf16, tag="et")
                        nc.scalar.activation(
                            et[:, :, q_off:], s[:, :, q_off:], AF.Exp, scale=0.125)
                        if kb * 128 >= jt * QT_TILE:  # diagonal block
                            nc.vector.tensor_tensor(
                                out=et[:, :, q_off:q_off + 128],
                                in0=et[:, :, q_off:q_off + 128],
                                in1=mask3[:],
                                op=OP.mult,
                            )
                        nc.tensor.matmul(
                            u[:, 0, q_off:], V[:, kb, 2 * hp, 0:65],
                            et[:, 0, q_off:],
                            start=(kb == 0), stop=(kb == nkb - 1),
                        )
                        nc.tensor.matmul(
                            u[:, 1, q_off:], V[:, kb, 2 * hp + 1, 0:65],
                            et[:, 1, q_off:],
                            start=(kb == 0), stop=(kb == nkb - 1),
                        )
                    # normalize: O^T = U^T * (1/rowsum), rowsum broadcast
                    # along partitions via one PE rank-2 matmul for both heads
                    sl = slice(jt * QT_TILE, (jt + 1) * QT_TILE)
                    rcp_e = rwpool.tile([1, 512], bf16, tag="rcp_e")
                    rcp_o = rwpool.tile([1, 512], bf16, tag="rcp_o")
                    with nc.allow_low_precision(reason="bf16 recip feeds bf16 bcast"):
                        nc.vector.reciprocal(rcp_e[:], u[64:65, 0, :])
                        nc.vector.reciprocal(rcp_o[:], u[64:65, 1, :])
                    # two rank-1 broadcasts into disjoint PE column groups
                    b_ps = ps_mm.tile([128, 512], f32, tag="mm512")
                    nc.tensor.matmul(
                        b_ps[0:64, :], ones[0:1, 0:64], rcp_e[:],
                        start=True, stop=True)
                    nc.tensor.matmul(
                        b_ps[64:128, :], ones[0:1, 0:64], rcp_o[:],
                        start=True, stop=True)
                    b_sb = rwpool.tile([128, 512], f32, tag="b_sb")
                    nc.scalar.copy(b_sb[:], b_ps[:])
                    nc.vector.tensor_tensor(
                        out=OT[0:64, hp, sl], in0=u[0:64, 0, :],
                        in1=b_sb[0:64, :], op=OP.mult)
                    nc.vector.tensor_tensor(
                        out=OT[64:128, hp, sl], in0=u[0:64, 1, :],
                        in1=b_sb[64:128, :], op=OP.mult)

            def emit_outproj(jt):
                for qb in range(jt * 4, (jt + 1) * 4):
                    for nb in range(D // 512):
                        y_ps = ps_mm.tile([128, 512], f32, tag="mm512")
                        for hp in range(4):
                            nc.tensor.matmul(
                                y_ps[:],
                                OT[:, hp, qb * 128:(qb + 1) * 128],
                                w2_sb[:, hp, nb * 512:(nb + 1) * 512],
                                start=(hp == 0),
                                stop=(hp == 3),
                            )
                        y_sb = ypool.tile([128, 512], bf16, tag="y_sb")
                        nc.vector.tensor_copy(y_sb[:], y_ps[:])
                        nc.sync.dma_start(
                            out[qb * 128:(qb + 1) * 128,
                                nb * 512:(nb + 1) * 512], y_sb[:]
                        )

            def emit_pass():
                phases = os.environ.get("KERNEL_PHASES", "ABC")
                for jt in range(L // 512):
                    if "A" in phases:
                        emit_proj(jt)
                    if "B" in phases:
                        emit_attn(jt)
                    if "C" in phases:
                        emit_outproj(jt)

            for _rep in range(repeat):
                emit_pass()
    nc.compile()
    return nc


def _get_nc():
    if "nc" not in _STATE:
        _STATE["nc"] = _build_nc()
    return _STATE["nc"]


def kernel(x, in_w, in_b, out_w, out_b):
    from concourse.bass_utils import run_bass_kernel_spmd

    bf = ml_dtypes.bfloat16
    x = np.asarray(x, dtype=np.float32)
    in_w = np.asarray(in_w, dtype=np.float32)
    in_b = np.asarray(in_b, dtype=np.float32)
    out_w = np.asarray(out_w, dtype=np.float32)
    out_b = np.asarray(out_b, dtype=np.float32)

    nc = _get_nc()

    in_maps = []
    for c in range(NCORES):
        b, hg = c // 2, c % 2
        hsl = slice(hg * HPC * HD, (hg + 1) * HPC * HD)  # 512 cols of each section
        wq = in_w[:, 0:D][:, hsl]
        wk = in_w[:, D:2 * D][:, hsl]
        wv_ = in_w[:, 2 * D:3 * D][:, hsl]
        bq = in_b[0:D][hsl]
        bk = in_b[D:2 * D][hsl]
        bv_ = in_b[2 * D:3 * D][hsl]
        in_maps.append({
            "xT": np.ascontiguousarray(x[b].T).astype(bf),
            "wqk": np.ascontiguousarray(
                np.concatenate([wq, wk], axis=1)).astype(bf),
            "wv": np.ascontiguousarray(wv_).astype(bf),
            "bqk": np.ascontiguousarray(
                np.concatenate([bq, bk]).reshape(8, 128).T).astype(np.float32),
            "bv": np.ascontiguousarray(bv_.reshape(1, -1)).astype(bf),
            "w2": np.ascontiguousarray(out_w[hsl, :]).astype(bf),
        })

    trace = bool(int(os.environ.get("KERNEL_TRACE", "0")))
    if not trace:
        # the axon NTFF profile hook is absent in this container; make sure a
        # stray BASS_TRACE=1 in the environment can't route us into it
        os.environ["BASS_NEVER_TRACE"] = "1"
    res = run_bass_kernel_spmd(
        nc, in_maps, core_ids=list(range(NCORES)), trace=trace,
    )
    _STATE["last_result"] = res
    _STATE["last_in_maps"] = in_maps

    y = np.zeros((B, L, D), dtype=np.float32)
    for c in range(NCORES):
        y[c // 2] += res.results[c]["out"].astype(np.float32)
    y += out_b[None, None, :]
    return y
